# revision 14
# baseline (speedup 1.0000x reference)
"""Bass/Trainium2 kernel for a 2-layer LSTM (B=512, T=2048, I=3, H=64).

Returns the final hidden state of layer 2, shape (512, 64) fp32.

Strategy (data-parallel over batch, 8 cores x 64 batch each):
  - All recurrent state lives in SBUF for the whole T=2048 recurrence.
  - State convention: ht = 2*h stored transposed (H on partitions, batch on
    free dim) in one (128, BL) fp16 tile: rows 0-63 = ht1 (layer1),
    rows 64-127 = ht2 (layer2).  Weights that multiply ht carry a 0.5.
  - sigmoid(z) = (tanh(z/2)+1)/2: the 0.5 is baked into the i/f/o gate
    weights, so ONE tanh ACTIVATE covers all four gates of a layer.
  - Cell state kept as c2x = 2*c in fp32; tanh(c) = tanh(0.5*c2x) via the
    ACT scale field.
  - x and the biases enter through a K=4 matmul (rows: x0,x1,x2,ones) from
    a host-pretransposed (4, T*BL) fp16 tensor, DMA'd in chunks.
  - The two layers run staggered by one timestep as two interleaved
    dependency chains.

Gate algebra per layer per step (i,f,g,o; ti=tanh(zi/2) etc, tg=tanh(zg)):
  u   = (ti + 1) * tg          # = 2*i*g            scalar_tensor_tensor
  w   = (0.5*c2x) * tf         # = tf*c             scalar_tensor_tensor
  s   = u + w                                        tensor_tensor
  c2x = 0.5*c2x + s            # = 2(f*c + i*g)     scalar_tensor_tensor
  tc  = tanh(0.5*c2x)                                ACT
  ht  = (to + 1) * tc          # = 2*o*tanh(c)      scalar_tensor_tensor
"""

import numpy as np

B, T, I, H = 512, 2048, 3, 64
NCORES = 8
BL = B // NCORES  # 64 batch per core
CH = 64  # timesteps per x-chunk DMA

_CACHE = {}


def _prep_weights(W_ih0, W_hh0, b_ih0, b_hh0, W_ih1, W_hh1, b_ih1, b_hh1):
    """Pack host-side lhsT weight arrays (fp16).

    Column order within each 256-col block: [i(64) | f(64) | g(64) | o(64)],
    i.e. if-block = cols 0..127, go-block = cols 128..255.
    """
    sg = np.concatenate(
        [np.full(H, 0.5), np.full(H, 0.5), np.full(H, 1.0), np.full(H, 0.5)]
    ).astype(np.float32)  # tanh-arg scale per gate row (i,f,g,o)

    b0 = (b_ih0 + b_hh0) * sg
    b1 = (b_ih1 + b_hh1) * sg
    Wx0 = W_ih0 * sg[:, None]  # acts on true x
    Wh0 = W_hh0 * sg[:, None] * 0.5  # acts on ht1 = 2*h1
    Wi1 = W_ih1 * sg[:, None] * 0.5  # acts on ht1
    Wh1 = W_hh1 * sg[:, None] * 0.5  # acts on ht2

    # Gate column order: layer 1 uses [f,i,o,g] so its elementwise algebra is
    # partition-aligned in rows 0-63; layer 2 uses [i,f,g,o] (aligned in rows
    # 64-127).  See cell_update.
    p1 = np.r_[H : 2 * H, 0:H, 3 * H : 4 * H, 2 * H : 3 * H]

    # w13: (68, 512).  cols 0-255: layer-1 lhsT (state rows 0-63, x rows
    # 64-66, bias row 67).  cols 256-511: layer-2 x-block lhsT (rows 64-66
    # zero, row 67 = layer-2 bias) -- rides the same K=4 rhs.
    w13 = np.zeros((68, 512), np.float32)
    w13[0:64, 0:256] = Wh0.T[:, p1]
    w13[64:67, 0:256] = Wx0.T[:, p1]
    w13[67, 0:256] = b0[p1]
    w13[67, 256:512] = b1
    # w2: (128, 256) layer-2 state lhsT: rows 0-63 act on ht1, 64-127 on ht2.
    w2 = np.concatenate([Wi1.T, Wh1.T], axis=0)
    return w13.astype(np.float16), np.ascontiguousarray(w2).astype(np.float16)


def build_program(t_steps=T, bl=BL):
    """Build the Bass program (one core's SPMD program)."""
    import concourse.bass as bass
    import concourse.tile as tile
    from concourse import bacc, mybir

    f32 = mybir.dt.float32
    f16 = mybir.dt.float16
    Tanh = mybir.ActivationFunctionType.Tanh
    ADD = mybir.AluOpType.add
    MULT = mybir.AluOpType.mult

    nc = bacc.Bacc("TRN2", target_bir_lowering=False, debug=False)

    xt_d = nc.dram_tensor("xt", [4, t_steps * bl], f16, kind="ExternalInput")
    w13_d = nc.dram_tensor("w13", [68, 512], f16, kind="ExternalInput")
    w2_d = nc.dram_tensor("w2", [128, 256], f16, kind="ExternalInput")
    out_d = nc.dram_tensor("out", [64, bl], f32, kind="ExternalOutput")

    n_chunks = (t_steps + CH - 1) // CH

    with tile.TileContext(nc) as tc:
        with (
            tc.tile_pool(name="const", bufs=1) as constp,
            tc.tile_pool(name="xchunk", bufs=2) as xpool,
            tc.tile_pool(name="gates", bufs=3) as gpool,
            tc.tile_pool(name="scratch", bufs=3) as spool,
            tc.tile_pool(name="ps1", bufs=2, space="PSUM") as ps1pool,
            tc.tile_pool(name="ps2", bufs=2, space="PSUM") as ps2pool,
        ):
            # --- constants / persistent state ---
            w13 = constp.tile([68, 512], f16, tag="w13")
            nc.sync.dma_start(w13[:, :], w13_d.ap()[:, :])
            w2 = constp.tile([128, 256], f16, tag="w2")
            nc.sync.dma_start(w2[:, :], w2_d.ap()[:, :])

            st = constp.tile([128, bl], f16, tag="state")  # [ht1; ht2]
            nc.vector.memset(st[:, :], 0.0)
            c1t = constp.tile([128, bl], f32, tag="c1")  # c2x layer1 (rows 0-63)
            nc.vector.memset(c1t[:, :], 0.0)
            c2t = constp.tile([128, bl], f32, tag="c2")  # c2x layer2 (rows 64-127)
            nc.vector.memset(c2t[:, :], 0.0)
            c1 = c1t[0:64, :]
            c2 = c2t[64:128, :]

            x_tiles = [None] * n_chunks

            def get_xchunk(ci):
                if x_tiles[ci] is None:
                    xt = xpool.tile([128, CH * bl], f16, tag="x")
                    lo = ci * CH * bl
                    hi = min((ci + 1) * CH, t_steps) * bl
                    nc.sync.dma_start(xt[64:68, 0 : hi - lo], xt_d.ap()[:, lo:hi])
                    x_tiles[ci] = xt
                return x_tiles[ci]

            def xslice(t):
                ci, off = divmod(t, CH)
                return get_xchunk(ci)[64:68, off * bl : (off + 1) * bl]

            # Per-layer step state handles
            ps2_of = {}  # step -> psum tile of layer-2 gates

            def l1_mms(t):
                """Layer-1 gate matmuls for step t -> psum (128, 2*bl)."""
                ps = ps1pool.tile([128, 512], f32, tag="ps1", name="ps1")[:, 0 : 2 * bl]
                xr = xslice(t)
                nc.tensor.matmul(ps[:, 0:bl], w13[64:68, 0:128], xr,
                                 start=True, stop=False)
                nc.tensor.matmul(ps[:, bl : 2 * bl], w13[64:68, 128:256], xr,
                                 start=False, stop=False)
                nc.tensor.matmul(ps[:, 0:bl], w13[0:64, 0:128], st[0:64, :],
                                 start=False, stop=False)
                nc.tensor.matmul(ps[:, bl : 2 * bl], w13[0:64, 128:256],
                                 st[0:64, :], start=False, stop=True)
                return ps

            def l2_mms(t):
                """Layer-2 gate matmuls for step t (needs ht1(t), ht2(t-1))."""
                ps = ps2pool.tile([128, 512], f32, tag="ps2", name="ps2")[:, 0 : 2 * bl]
                xr = xslice(t)  # only the ones-row matters (rows 64-66 hit zeros)
                nc.tensor.matmul(ps[:, 0:bl], w13[64:68, 256:384], xr,
                                 start=True, stop=False)
                nc.tensor.matmul(ps[:, bl : 2 * bl], w13[64:68, 384:512], xr,
                                 start=False, stop=False)
                nc.tensor.matmul(ps[:, 0:bl], w2[:, 0:128], st[:, :],
                                 start=False, stop=False)
                nc.tensor.matmul(ps[:, bl : 2 * bl], w2[:, 128:256], st[:, :],
                                 start=False, stop=True)
                ps2_of[t] = ps

            def slices_of(t1, layer):
                """Layer 1 gate col order [f,i,o,g]: algebra rows 0-63.
                Layer 2 gate col order [i,f,g,o]: algebra rows 64-127."""
                if layer == 1:
                    lo = slice(0, 64)
                    tf, ti = t1[0:64, 0:bl], t1[64:128, 0:bl]
                    to, tg = t1[0:64, bl : 2 * bl], t1[64:128, bl : 2 * bl]
                else:
                    lo = slice(64, 128)
                    ti, tf = t1[0:64, 0:bl], t1[64:128, 0:bl]
                    tg, to = t1[0:64, bl : 2 * bl], t1[64:128, bl : 2 * bl]
                return lo, ti, tf, tg, to

            def cell_a(ps, layer):
                """ACT: tanh over all four gate blocks."""
                t1 = gpool.tile([128, 2 * bl], f16, tag=f"t1l{layer}",
                                name=f"t1l{layer}")
                nc.scalar.activation(t1[:, :], ps[:, :], Tanh)
                return t1

            def cell_b(t1, cc, layer):
                """DVE: c2x = 0.5*c2x + (ti+1)*tg + tf*(0.5*c2x)."""
                lo, ti, tf, tg, to = slices_of(t1, layer)
                u = spool.tile([128, bl], f16, tag=f"u{layer}", name=f"u{layer}")[lo, :]
                nc.vector.scalar_tensor_tensor(u, ti, 1.0, tg, ADD, MULT)
                w = spool.tile([128, bl], f32, tag=f"w{layer}", name=f"w{layer}")[lo, :]
                nc.vector.scalar_tensor_tensor(w, cc, 0.5, tf, MULT, MULT)
                s = spool.tile([128, bl], f32, tag=f"s{layer}", name=f"s{layer}")[lo, :]
                nc.vector.tensor_tensor(s, u, w, ADD)
                nc.vector.scalar_tensor_tensor(cc, cc, 0.5, s, MULT, ADD)

            def cell_c(t1, cc, layer):
                """ACT tanh(c) + DVE ht = (to+1)*tc -> st."""
                lo, ti, tf, tg, to = slices_of(t1, layer)
                tcl = spool.tile([128, bl], f16, tag=f"tc{layer}",
                                 name=f"tc{layer}")[lo, :]
                nc.scalar.activation(tcl, cc, Tanh, scale=0.5)
                nc.vector.scalar_tensor_tensor(st[lo, :], to, 1.0, tcl, ADD, MULT)

            # Emission order = per-engine queue order.  Interleave the two
            # layer chains (L2 runs one step behind L1) so neither chain
            # head-of-line-blocks the other on the ACT/DVE FIFOs.
            for t in range(t_steps):
                if t >= 1:
                    l2_mms(t - 1)
                ps1 = l1_mms(t)
                t1b = cell_a(ps2_of.pop(t - 1), 2) if t >= 1 else None
                t1a = cell_a(ps1, 1)
                if t1b is not None:
                    cell_b(t1b, c2, 2)
                cell_b(t1a, c1, 1)
                if t1b is not None:
                    cell_c(t1b, c2, 2)  # writes ht2(t-1)
                cell_c(t1a, c1, 1)  # writes ht1(t)
                # free old x chunk handle (keeps python refs bounded)
                ci = t // CH
                if ci >= 2:
                    x_tiles[ci - 2] = None

            l2_mms(t_steps - 1)
            t1b = cell_a(ps2_of.pop(t_steps - 1), 2)
            cell_b(t1b, c2, 2)
            cell_c(t1b, c2, 2)

            # out = 0.5 * ht2 = h2_final (transposed: H x batch), fp32
            ob = constp.tile([128, bl], f32, tag="out")
            nc.vector.tensor_scalar_mul(ob[64:128, :], st[64:128, :], 0.5)
            nc.sync.dma_start(out_d.ap()[:, :], ob[64:128, :])

    nc.compile()
    return nc


def _get_program(t_steps=T):
    key = ("prog", t_steps)
    if key not in _CACHE:
        _CACHE[key] = build_program(t_steps)
    return _CACHE[key]


def kernel(x, W_ih0, W_hh0, b_ih0, b_hh0, W_ih1, W_hh1, b_ih1, b_hh1):
    from concourse import bass_utils

    x = np.asarray(x, np.float32)
    w13, w2 = _prep_weights(
        np.asarray(W_ih0, np.float32), np.asarray(W_hh0, np.float32),
        np.asarray(b_ih0, np.float32), np.asarray(b_hh0, np.float32),
        np.asarray(W_ih1, np.float32), np.asarray(W_hh1, np.float32),
        np.asarray(b_ih1, np.float32), np.asarray(b_hh1, np.float32),
    )

    nc = _get_program(T)

    in_maps = []
    for c in range(NCORES):
        xc = x[c * BL : (c + 1) * BL]  # (BL, T, 3)
        xt = np.ones((4, T * BL), np.float16)
        xt[0:3] = xc.transpose(2, 1, 0).reshape(3, T * BL).astype(np.float16)
        in_maps.append({"xt": xt, "w13": w13, "w2": w2})

    res = bass_utils.run_bass_kernel_spmd(nc, in_maps, core_ids=list(range(NCORES)))
    outs = [res.results[c]["out"].T for c in range(NCORES)]  # (BL, 64) each
    return np.concatenate(outs, axis=0).astype(np.float32)


if __name__ == "__main__":
    rng = np.random.default_rng(0)
    s = 1.0 / np.sqrt(H)
    inputs = {
        "x": rng.standard_normal((B, T, I), np.float32),
        "W_ih0": rng.uniform(-s, s, (4 * H, I)).astype(np.float32),
        "W_hh0": rng.uniform(-s, s, (4 * H, H)).astype(np.float32),
        "b_ih0": rng.uniform(-s, s, 4 * H).astype(np.float32),
        "b_hh0": rng.uniform(-s, s, 4 * H).astype(np.float32),
        "W_ih1": rng.uniform(-s, s, (4 * H, H)).astype(np.float32),
        "W_hh1": rng.uniform(-s, s, (4 * H, H)).astype(np.float32),
        "b_ih1": rng.uniform(-s, s, 4 * H).astype(np.float32),
        "b_hh1": rng.uniform(-s, s, 4 * H).astype(np.float32),
    }
    out = kernel(**inputs)
    print(out.shape, out.dtype, np.abs(out).max())
